# revision 8
# baseline (speedup 1.0000x reference)
"""Trainium2 Bass kernel for nn_ClusterLoss (vq_codebook).

reference:
    f = l2norm(features); c = l2norm(centers)
    sims = f @ c.T ; a = argmax(sims, -1)
    loss = mean(sum((f - centers[a])**2, -1))

Device algorithm (per core, data-parallel over N):
    G = beta * (f_bf16 @ c_hat_bf16.T)            # PE, PSUM f32 [128,1024] per tile
    negbm = -max_k(G)                             # DVE reduce_max(negate)
    E = exp(G + negbm), Z = sum_k E               # ACT, per-partition bias, fused accum
    S = sum_k E * ||c_k||                         # DVE fused tensor_tensor_reduce
    r* = S/Z  (== ||c_argmax|| since E is a numerically-exact one-hot as beta->inf)
    loss_row = 1 - 2*(max G/beta)*r*/||f|| + r*^2
Host sums per-row losses (f64) across cores / N.

Identity used: ||f_hat - c_a||^2 = 1 - 2*s*||c_a|| + ||c_a||^2 with
s = cos-sim = max_k G / (beta*||f||); argmax over unnormalized f is the same.
"""
import sys

sys.path.insert(0, "/opt/trn_rl_repo")

from contextlib import ExitStack

import numpy as np

import concourse.bass as bass
import concourse.bacc as bacc
import concourse.mybir as mybir
from concourse import tile
from concourse.bass_utils import run_bass_kernel_spmd

F32 = mybir.dt.float32
BF16 = mybir.dt.bfloat16
NP_BF16 = mybir.dt.np(mybir.dt.bfloat16)

N_CORES = 8
N_TOTAL = 131072
D = 128
K = 1024
ROWS_PER_CORE = N_TOTAL // N_CORES
BETA = 16384.0  # power of two: bf16(beta*c_hat) == beta*bf16(c_hat)

_nc_cache = {}


def build_nc(rows_per_core=ROWS_PER_CORE):
    return build_nc_rep(rows_per_core, rep=1)


def build_nc_rep(rows_per_core=ROWS_PER_CORE, rep=1):
    """Build + compile the per-core bass program (SPMD across 8 cores).

    rep>1 repeats the (idempotent) main loop for wall-clock HW timing.
    """
    if (rows_per_core, rep) in _nc_cache:
        return _nc_cache[(rows_per_core, rep)]

    R = rows_per_core
    T = R // 128  # number of 128-row tiles
    KT = K // 128  # center tiles (8)

    nc = bacc.Bacc("TRN2", target_bir_lowering=False, debug=False, num_devices=N_CORES)

    ft = nc.dram_tensor("ft", [128, R], BF16, kind="ExternalInput").ap()  # f^T
    fn = nc.dram_tensor("fn", [R, 128], BF16, kind="ExternalInput").ap()  # f natural
    cn = nc.dram_tensor("cn", [K, 128], F32, kind="ExternalInput").ap()  # centers
    ident = nc.dram_tensor("ident", [128, 128], BF16, kind="ExternalInput").ap()
    ones1 = nc.dram_tensor("ones1", [1, 128], BF16, kind="ExternalInput").ap()
    lossw = nc.dram_tensor("lossw", [128, T], F32, kind="ExternalOutput").ap()

    with tile.TileContext(nc) as tc, ExitStack() as ctx:
        const = ctx.enter_context(tc.tile_pool(name="const", bufs=1))
        setup = ctx.enter_context(tc.tile_pool(name="setup", bufs=2))
        setup_ps_cm = tc.tile_pool(name="setup_ps", bufs=1, space="PSUM")
        setup_ps = setup_ps_cm.__enter__()
        epool = ctx.enter_context(tc.tile_pool(name="epool", bufs=3))
        trash = ctx.enter_context(tc.tile_pool(name="trash", bufs=2))

        # ---------------- constants / big input loads ----------------
        ft_sb = const.tile([128, R], BF16)
        nc.sync.dma_start(ft_sb[:], ft)

        fn_sb = const.tile([128, T * 128], BF16)
        fn_v = fn_sb[:].rearrange("p (t d) -> p t d", d=128)
        nc.sync.dma_start(fn_v, fn.rearrange("(t p) d -> p t d", p=128))

        id_sb = const.tile([128, 128], BF16)
        nc.sync.dma_start(id_sb[:], ident)
        on_sb = const.tile([1, 128], BF16)
        nc.sync.dma_start(on_sb[:], ones1)

        ct_sb = const.tile([128, KT * 128], F32)  # natural centers [kpart, (j d)]
        ct_v = ct_sb[:].rearrange("p (j d) -> p j d", d=128)
        nc.sync.dma_start(ct_v, cn.rearrange("(j p) d -> p j d", p=128))

        # ---------------- center setup ----------------
        # q[p, j] = ||c_{j*128+p}||^2
        qw = setup.tile([128, KT], F32)
        for j in range(KT):
            sq_t = trash.tile([128, 128], F32)
            nc.scalar.activation(
                sq_t[:], ct_v[:, j, :], mybir.ActivationFunctionType.Square,
                accum_out=qw[:, j : j + 1],
            )
        # rinv = 1/||c||  (reciprocal then sqrt)
        qinv = setup.tile([128, KT], F32)
        nc.vector.reciprocal(qinv[:], qw[:])
        rinv = setup.tile([128, KT], F32)
        nc.scalar.activation(rinv[:], qinv[:], mybir.ActivationFunctionType.Sqrt)
        # r = q * rinv = ||c||
        rr = setup.tile([128, KT], F32)
        nc.vector.tensor_mul(rr[:], qw[:], rinv[:])
        rr_bf = setup.tile([128, KT], BF16)
        nc.vector.tensor_copy(rr_bf[:], rr[:])
        # scaled normalizer: beta / ||c||
        rinv_b = setup.tile([128, KT], F32)
        nc.vector.tensor_scalar_mul(rinv_b[:], rinv[:], float(BETA))

        # c_hat_scaled tiles (natural layout) then PE-transpose into chT [d, k]
        chT_sb = const.tile([128, K], BF16)
        for j in range(KT):
            ch_j = setup.tile([128, 128], BF16, tag="ch_j")
            nc.vector.tensor_scalar_mul(ch_j[:], ct_v[:, j, :], rinv_b[:, j : j + 1])
            chT_ps = setup_ps.tile([128, 128], BF16, tag="chT_ps")
            nc.tensor.transpose(chT_ps[:], ch_j[:], id_sb[:])
            nc.scalar.activation(
                chT_sb[:, j * 128 : (j + 1) * 128], chT_ps[:],
                mybir.ActivationFunctionType.Copy,
            )

        # r broadcast across partitions: rb[p, k] = ||c_k|| for all p.
        # Column-wise PE transposes build r_row [1, K] at partition 0,
        # then ones (x) r_row broadcasts it to all partitions.
        r_row = setup.tile([1, K], BF16)
        for j in range(KT):
            rt_ps = setup_ps.tile([1, 128], BF16, tag="rt_ps")
            nc.tensor.transpose(rt_ps[:], rr_bf[:, j : j + 1], id_sb[:])
            nc.scalar.activation(
                r_row[:, j * 128 : (j + 1) * 128], rt_ps[:],
                mybir.ActivationFunctionType.Copy,
            )
        rb_ps = setup_ps.tile([128, K], F32, tag="rb_ps")
        nc.tensor.matmul(rb_ps[:, 0:512], on_sb[:], r_row[:, 0:512], start=True, stop=True)
        nc.tensor.matmul(rb_ps[:, 512:1024], on_sb[:], r_row[:, 512:1024], start=True, stop=True)
        rb_sb = const.tile([128, K], BF16)
        nc.scalar.activation(rb_sb[:], rb_ps[:], mybir.ActivationFunctionType.Copy)

        setup_ps_cm.__exit__(None, None, None)
        gpool = ctx.enter_context(tc.tile_pool(name="gpool", bufs=3, space="PSUM"))

        # ---------------- wide accumulators ----------------
        negbm_w = const.tile([128, T], F32)
        zw = const.tile([128, T], F32)
        sw = const.tile([128, T], F32)
        n2w = const.tile([128, T], F32)

        # ---------------- main loop ----------------
        for t in [t for _ in range(rep) for t in range(T)]:
            g_ps = gpool.tile([128, K], F32)
            lhs = ft_sb[:, t * 128 : (t + 1) * 128]
            nc.tensor.matmul(g_ps[:, 0:512], lhs, chT_sb[:, 0:512], start=True, stop=True)
            nc.tensor.matmul(g_ps[:, 512:1024], lhs, chT_sb[:, 512:1024], start=True, stop=True)

            nc.vector.reduce_max(
                negbm_w[:, t : t + 1], g_ps[:], axis=mybir.AxisListType.X, negate=True
            )
            e_sb = epool.tile([128, K], BF16)
            nc.scalar.activation(
                e_sb[:], g_ps[:], mybir.ActivationFunctionType.Exp,
                bias=negbm_w[:, t : t + 1], scale=1.0,
                accum_out=zw[:, t : t + 1],
            )
            tr = trash.tile([128, K], BF16, tag="ttr_out")
            nc.vector.tensor_mul(tr[:], e_sb[:], rb_sb[:])
            nc.vector.reduce_sum(
                sw[:, t : t + 1], tr[:], axis=mybir.AxisListType.X
            )
            sqf = trash.tile([128, 128], BF16, tag="sqf")
            nc.scalar.activation(
                sqf[:], fn_v[:, t, :], mybir.ActivationFunctionType.Square,
                accum_out=n2w[:, t : t + 1],
            )

        # ---------------- epilogue: per-row loss ----------------
        m_w = setup.tile([128, T], F32)
        nc.vector.tensor_scalar_mul(m_w[:], negbm_w[:], -1.0 / BETA)
        n2i = setup.tile([128, T], F32)
        nc.vector.reciprocal(n2i[:], n2w[:])
        invn = setup.tile([128, T], F32)
        nc.scalar.activation(invn[:], n2i[:], mybir.ActivationFunctionType.Sqrt)
        zi = setup.tile([128, T], F32)
        nc.vector.reciprocal(zi[:], zw[:])
        rstar = setup.tile([128, T], F32)
        nc.vector.tensor_mul(rstar[:], sw[:], zi[:])
        a_w = setup.tile([128, T], F32)
        nc.vector.tensor_mul(a_w[:], m_w[:], invn[:])
        b_w = setup.tile([128, T], F32)
        nc.vector.tensor_mul(b_w[:], a_w[:], rstar[:])
        b2_w = setup.tile([128, T], F32)
        nc.vector.tensor_scalar_mul(b2_w[:], b_w[:], -2.0)
        r2_w = setup.tile([128, T], F32)
        nc.vector.tensor_mul(r2_w[:], rstar[:], rstar[:])
        t3_w = setup.tile([128, T], F32)
        nc.vector.tensor_add(t3_w[:], r2_w[:], b2_w[:])
        lw = setup.tile([128, T], F32)
        nc.vector.tensor_scalar_add(lw[:], t3_w[:], 1.0)
        nc.sync.dma_start(lossw, lw[:])

    nc.compile()
    _nc_cache[(rows_per_core, rep)] = nc
    return nc


def make_in_maps(features, centers, rows_per_core=ROWS_PER_CORE, n_cores=N_CORES):
    f_bf = features.astype(NP_BF16)
    shards = f_bf.reshape(n_cores, rows_per_core, D)
    ident = np.eye(128, dtype=NP_BF16)
    ones1 = np.ones((1, 128), dtype=NP_BF16)
    cns = np.ascontiguousarray(centers.astype(np.float32))
    in_maps = []
    for c in range(n_cores):
        s = shards[c]
        in_maps.append(
            {
                "ft": np.ascontiguousarray(s.T),
                "fn": np.ascontiguousarray(s),
                "cn": cns,
                "ident": ident,
                "ones1": ones1,
            }
        )
    return in_maps


def kernel(features, centers):
    features = np.asarray(features)
    centers = np.asarray(centers)
    nc = build_nc(ROWS_PER_CORE)
    in_maps = make_in_maps(features, centers)
    res = run_bass_kernel_spmd(nc, in_maps, core_ids=list(range(N_CORES)))
    total = 0.0
    for c in range(N_CORES):
        total += res.results[c]["lossw"].astype(np.float64).sum()
    return np.float32(total / (ROWS_PER_CORE * N_CORES))


# revision 12
# speedup vs baseline: 339.0817x; 339.0817x over previous
"""Trainium2 Bass kernel for nn_ClusterLoss (vq_codebook).

reference:
    f = l2norm(features); c = l2norm(centers)
    sims = f @ c.T ; a = argmax(sims, -1)
    loss = mean(sum((f - centers[a])**2, -1))

Device algorithm (per core, data-parallel over N, 16384 rows each):
  per 128-row tile (PSUM tile [128,1024] f32):
    G  = beta * (f_bf16 @ c_hat_bf16.T)      # PE (2 matmuls, free=512)
    negbm = -max_k G                          # DVE reduce_max(negate) - only DVE pass
    G += ln||c_k||  (rank-1 accumulate)       # PE (2 K=1 matmuls, after the max read)
    E  = exp(G + negbm);  S = sum_k E         # ACT, fused accum_out
       = sum_k ||c_k|| * e^{beta(G_k - m)}  ~= ||c_argmax||   (exact one-hot as beta->inf)
    ||f||^2 per row                           # GPSIMD square+reduce (otherwise idle)
  per-row loss = 1 - 2*(m/beta)*S/||f|| + S^2 ; host sums losses (f64) across cores.

Identity: ||f_hat - c_a||^2 = 1 - 2*cos*||c_a|| + ||c_a||^2; argmax of cos-sim
is invariant to scaling by beta/||f||, so raw bf16 features feed the matmul.
"""
import sys

sys.path.insert(0, "/opt/trn_rl_repo")

from contextlib import ExitStack

import numpy as np

import concourse.bass as bass
import concourse.bacc as bacc
import concourse.mybir as mybir
from concourse import tile
from concourse.bass_utils import run_bass_kernel_spmd

F32 = mybir.dt.float32
BF16 = mybir.dt.bfloat16
NP_BF16 = mybir.dt.np(mybir.dt.bfloat16)
AF = mybir.ActivationFunctionType
AX = mybir.AxisListType

N_CORES = 8
N_TOTAL = 131072
D = 128
K = 1024
ROWS_PER_CORE = N_TOTAL // N_CORES
BETA = 32768.0  # power of two: bf16(beta*c_hat) == beta*bf16(c_hat)

_nc_cache = {}


def build_nc(rows_per_core=ROWS_PER_CORE):
    return build_nc_rep(rows_per_core, rep=1)


def build_nc_rep(rows_per_core=ROWS_PER_CORE, rep=1):
    """Build + compile the per-core bass program (SPMD across 8 cores).

    rep>1 wraps the (idempotent) main loop in a hardware For_i loop --
    used for wall-clock HW timing with a constant-size NEFF.
    """
    if (rows_per_core, rep) in _nc_cache:
        return _nc_cache[(rows_per_core, rep)]

    R = rows_per_core
    T = R // 128  # number of 128-row tiles
    KT = K // 128  # center tiles (8)

    nc = bacc.Bacc("TRN2", target_bir_lowering=False, debug=False, num_devices=N_CORES)

    ft = nc.dram_tensor("ft", [128, R], BF16, kind="ExternalInput").ap()  # f^T
    fn = nc.dram_tensor("fn", [R, 128], BF16, kind="ExternalInput").ap()  # f natural
    cn = nc.dram_tensor("cn", [K, 128], F32, kind="ExternalInput").ap()  # centers
    ident = nc.dram_tensor("ident", [128, 128], BF16, kind="ExternalInput").ap()
    ones1 = nc.dram_tensor("ones1", [1, 128], BF16, kind="ExternalInput").ap()
    lossw = nc.dram_tensor("lossw", [128, T], F32, kind="ExternalOutput").ap()

    with tile.TileContext(nc) as tc, ExitStack() as ctx:
        const = ctx.enter_context(tc.tile_pool(name="const", bufs=1))
        setup = ctx.enter_context(tc.tile_pool(name="setup", bufs=2))
        setup_ps_cm = tc.tile_pool(name="setup_ps", bufs=1, space="PSUM")
        setup_ps = setup_ps_cm.__enter__()
        epool = ctx.enter_context(tc.tile_pool(name="epool", bufs=3))
        trash = ctx.enter_context(tc.tile_pool(name="trash", bufs=2))

        # ---------------- constants / big input loads ----------------
        ft_sb = const.tile([128, R], BF16)
        nc.sync.dma_start(ft_sb[:], ft)

        fn_sb = const.tile([128, T * 128], BF16)
        fn_v = fn_sb[:].rearrange("p (t d) -> p t d", d=128)
        nc.sync.dma_start(fn_v, fn.rearrange("(t p) d -> p t d", p=128))

        id_sb = const.tile([128, 128], BF16)
        nc.sync.dma_start(id_sb[:], ident)
        on_sb = const.tile([1, 128], BF16)
        nc.sync.dma_start(on_sb[:], ones1)

        ct_sb = const.tile([128, KT * 128], F32)  # natural centers [kpart, (j d)]
        ct_v = ct_sb[:].rearrange("p (j d) -> p j d", d=128)
        nc.sync.dma_start(ct_v, cn.rearrange("(j p) d -> p j d", p=128))

        # ---------------- center setup ----------------
        # q[p, j] = ||c_{j*128+p}||^2
        qw = setup.tile([128, KT], F32)
        for j in range(KT):
            sq_t = trash.tile([128, 128], F32, tag="sq_t")
            nc.scalar.activation(
                sq_t[:], ct_v[:, j, :], AF.Square, accum_out=qw[:, j : j + 1]
            )
        # rinv = 1/||c||  (reciprocal then sqrt)
        qinv = setup.tile([128, KT], F32)
        nc.vector.reciprocal(qinv[:], qw[:])
        rinv = setup.tile([128, KT], F32)
        nc.scalar.activation(rinv[:], qinv[:], AF.Sqrt)
        # r = q * rinv = ||c||;  lnr = ln ||c||
        rr = setup.tile([128, KT], F32)
        nc.vector.tensor_mul(rr[:], qw[:], rinv[:])
        lnr = setup.tile([128, KT], F32)
        nc.scalar.activation(lnr[:], rr[:], AF.Ln)
        lnr_bf = setup.tile([128, KT], BF16)
        nc.vector.tensor_copy(lnr_bf[:], lnr[:])
        # scaled normalizer: beta / ||c||
        rinv_b = setup.tile([128, KT], F32)
        nc.vector.tensor_scalar_mul(rinv_b[:], rinv[:], float(BETA))

        # c_hat_scaled tiles (natural layout) then PE-transpose into chT [d, k]
        chT_sb = const.tile([128, K], BF16)
        for j in range(KT):
            ch_j = setup.tile([128, 128], BF16, tag="ch_j")
            nc.vector.tensor_scalar_mul(ch_j[:], ct_v[:, j, :], rinv_b[:, j : j + 1])
            chT_ps = setup_ps.tile([128, 128], BF16, tag="chT_ps")
            nc.tensor.transpose(chT_ps[:], ch_j[:], id_sb[:])
            nc.scalar.activation(
                chT_sb[:, j * 128 : (j + 1) * 128], chT_ps[:], AF.Copy
            )

        # lnr_row [1, K] at partition 0 via column-wise PE transposes
        lnr_row = const.tile([1, K], BF16)
        for j in range(KT):
            rt_ps = setup_ps.tile([1, 128], BF16, tag="rt_ps")
            nc.tensor.transpose(rt_ps[:], lnr_bf[:, j : j + 1], id_sb[:])
            nc.scalar.activation(
                lnr_row[:, j * 128 : (j + 1) * 128], rt_ps[:], AF.Copy
            )

        setup_ps_cm.__exit__(None, None, None)
        gpool = ctx.enter_context(tc.tile_pool(name="gpool", bufs=3, space="PSUM"))

        # ---------------- wide accumulators ----------------
        negbm_w = const.tile([128, T], F32)
        sw = const.tile([128, T], F32)
        n2w = const.tile([128, T], F32)

        # ---------------- main loop (1-tile software pipeline) ----------------
        GN = 8  # norm-reduce batching group

        def emit_head(t, sq8):
            g_ps = gpool.tile([128, K], F32)
            lhs = ft_sb[:, t * 128 : (t + 1) * 128]
            nc.tensor.matmul(g_ps[:, 0:512], lhs, chT_sb[:, 0:512], start=True, stop=True)
            nc.tensor.matmul(g_ps[:, 512:1024], lhs, chT_sb[:, 512:1024], start=True, stop=True)
            nc.vector.reduce_max(
                negbm_w[:, t : t + 1], g_ps[:], axis=AX.X, negate=True
            )
            # ||f||^2 partial: square on the (otherwise idle) GPSIMD engine
            nc.gpsimd.tensor_mul(
                sq8[:, (t % GN) * 128 : (t % GN + 1) * 128],
                fn_v[:, t, :], fn_v[:, t, :],
            )
            return g_ps

        def emit_tail(t, g_ps):
            nc.tensor.matmul(g_ps[:, 0:512], on_sb[:], lnr_row[:, 0:512], start=False, stop=True, skip_group_check=True)
            nc.tensor.matmul(g_ps[:, 512:1024], on_sb[:], lnr_row[:, 512:1024], start=False, stop=True, skip_group_check=True)
            e_sb = epool.tile([128, K], BF16)
            nc.scalar.activation(
                e_sb[:], g_ps[:], AF.Exp,
                bias=negbm_w[:, t : t + 1], scale=1.0,
                accum_out=sw[:, t : t + 1],
            )

        def one_pass(_i=None):
            prev = None
            for g0 in range(0, T, GN):
                gn = min(GN, T - g0)
                sq8 = trash.tile([128, GN * 128], F32, tag="sq8")
                for t in range(g0, g0 + gn):
                    g_ps = emit_head(t, sq8)
                    if prev is not None:
                        emit_tail(*prev)
                    prev = (t, g_ps)
                nc.vector.reduce_sum(
                    n2w[:, g0 : g0 + gn],
                    sq8[:].rearrange("p (g d) -> p g d", d=128)[:, 0:gn, :],
                    axis=AX.X,
                )
            emit_tail(*prev)

        if rep == 1:
            one_pass()
        else:
            with tc.For_i(0, rep) as _i:
                one_pass(_i)

        # ---------------- epilogue: per-row loss ----------------
        m_w = setup.tile([128, T], F32)
        nc.vector.tensor_scalar_mul(m_w[:], negbm_w[:], -1.0 / BETA)
        n2i = setup.tile([128, T], F32)
        nc.vector.reciprocal(n2i[:], n2w[:])
        invn = setup.tile([128, T], F32)
        nc.scalar.activation(invn[:], n2i[:], AF.Sqrt)
        a_w = setup.tile([128, T], F32)
        nc.vector.tensor_mul(a_w[:], m_w[:], invn[:])
        b_w = setup.tile([128, T], F32)
        nc.vector.tensor_mul(b_w[:], a_w[:], sw[:])
        b2_w = setup.tile([128, T], F32)
        nc.vector.tensor_scalar_mul(b2_w[:], b_w[:], -2.0)
        r2_w = setup.tile([128, T], F32)
        nc.vector.tensor_mul(r2_w[:], sw[:], sw[:])
        t3_w = setup.tile([128, T], F32)
        nc.vector.tensor_add(t3_w[:], r2_w[:], b2_w[:])
        lw = setup.tile([128, T], F32)
        nc.vector.tensor_scalar_add(lw[:], t3_w[:], 1.0)
        nc.sync.dma_start(lossw, lw[:])

    nc.compile()
    _nc_cache[(rows_per_core, rep)] = nc
    return nc


def make_in_maps(features, centers, rows_per_core=ROWS_PER_CORE, n_cores=N_CORES):
    f_bf = features.astype(NP_BF16)
    shards = f_bf.reshape(n_cores, rows_per_core, D)
    ident = np.eye(128, dtype=NP_BF16)
    ones1 = np.ones((1, 128), dtype=NP_BF16)
    cns = np.ascontiguousarray(centers.astype(np.float32))
    in_maps = []
    for c in range(n_cores):
        s = shards[c]
        in_maps.append(
            {
                "ft": np.ascontiguousarray(s.T),
                "fn": np.ascontiguousarray(s),
                "cn": cns,
                "ident": ident,
                "ones1": ones1,
            }
        )
    return in_maps


def kernel(features, centers):
    features = np.asarray(features)
    centers = np.asarray(centers)
    nc = build_nc(ROWS_PER_CORE)
    in_maps = make_in_maps(features, centers)
    res = run_bass_kernel_spmd(nc, in_maps, core_ids=list(range(N_CORES)))
    total = 0.0
    for c in range(N_CORES):
        total += res.results[c]["lossw"].astype(np.float64).sum()
    return np.float32(total / (ROWS_PER_CORE * N_CORES))


# revision 13
# speedup vs baseline: 357.1823x; 1.0534x over previous
"""Trainium2 Bass kernel for nn_ClusterLoss (vq_codebook).

reference:
    f = l2norm(features); c = l2norm(centers)
    sims = f @ c.T ; a = argmax(sims, -1)
    loss = mean(sum((f - centers[a])**2, -1))

Device algorithm (per core, data-parallel over N, 16384 rows each):
  per 128-row tile (PSUM tile [128,1024] f32):
    G  = beta * (f_bf16 @ c_hat_bf16.T)      # PE (2 matmuls, free=512)
    negbm = -max_k G                          # DVE reduce_max(negate) - only DVE pass
    G += ln||c_k||  (rank-1 accumulate)       # PE (2 K=1 matmuls, after the max read)
    E  = exp(G + negbm);  S = sum_k E         # ACT, fused accum_out
       = sum_k ||c_k|| * e^{beta(G_k - m)}  ~= ||c_argmax||   (exact one-hot as beta->inf)
    ||f||^2 per row                           # GPSIMD square+reduce (otherwise idle)
  per-row loss = 1 - 2*(m/beta)*S/||f|| + S^2 ; host sums losses (f64) across cores.

Identity: ||f_hat - c_a||^2 = 1 - 2*cos*||c_a|| + ||c_a||^2; argmax of cos-sim
is invariant to scaling by beta/||f||, so raw bf16 features feed the matmul.
"""
import sys

sys.path.insert(0, "/opt/trn_rl_repo")

from contextlib import ExitStack

import numpy as np

import concourse.bass as bass
import concourse.bacc as bacc
import concourse.mybir as mybir
from concourse import tile
from concourse.bass_utils import run_bass_kernel_spmd

F32 = mybir.dt.float32
BF16 = mybir.dt.bfloat16
NP_BF16 = mybir.dt.np(mybir.dt.bfloat16)
AF = mybir.ActivationFunctionType
AX = mybir.AxisListType

N_CORES = 8
N_TOTAL = 131072
D = 128
K = 1024
ROWS_PER_CORE = N_TOTAL // N_CORES
BETA = 32768.0  # power of two: bf16(beta*c_hat) == beta*bf16(c_hat)

_nc_cache = {}


def build_nc(rows_per_core=ROWS_PER_CORE):
    return build_nc_rep(rows_per_core, rep=1)


def build_nc_rep(rows_per_core=ROWS_PER_CORE, rep=1):
    """Build + compile the per-core bass program (SPMD across 8 cores).

    rep>1 wraps the (idempotent) main loop in a hardware For_i loop --
    used for wall-clock HW timing with a constant-size NEFF.
    """
    if (rows_per_core, rep) in _nc_cache:
        return _nc_cache[(rows_per_core, rep)]

    R = rows_per_core
    T = R // 128  # number of 128-row tiles
    KT = K // 128  # center tiles (8)

    nc = bacc.Bacc("TRN2", target_bir_lowering=False, debug=False, num_devices=N_CORES)

    ft = nc.dram_tensor("ft", [128, R], BF16, kind="ExternalInput").ap()  # f^T
    fn = nc.dram_tensor("fn", [R, 128], BF16, kind="ExternalInput").ap()  # f natural
    cn = nc.dram_tensor("cn", [K, 128], F32, kind="ExternalInput").ap()  # centers
    ident = nc.dram_tensor("ident", [128, 128], BF16, kind="ExternalInput").ap()
    ones1 = nc.dram_tensor("ones1", [1, 128], BF16, kind="ExternalInput").ap()
    lossw = nc.dram_tensor("lossw", [128, T], F32, kind="ExternalOutput").ap()

    with tile.TileContext(nc) as tc, ExitStack() as ctx:
        const = ctx.enter_context(tc.tile_pool(name="const", bufs=1))
        setup = ctx.enter_context(tc.tile_pool(name="setup", bufs=2))
        setup_ps_cm = tc.tile_pool(name="setup_ps", bufs=1, space="PSUM")
        setup_ps = setup_ps_cm.__enter__()
        epool = ctx.enter_context(tc.tile_pool(name="epool", bufs=3))
        bpool = ctx.enter_context(tc.tile_pool(name="bpool", bufs=6))
        trash = ctx.enter_context(tc.tile_pool(name="trash", bufs=2))

        # ---------------- constants / big input loads ----------------
        ft_sb = const.tile([128, R], BF16)
        nc.sync.dma_start(ft_sb[:], ft)

        fn_sb = const.tile([128, T * 128], BF16)
        fn_v = fn_sb[:].rearrange("p (t d) -> p t d", d=128)
        nc.sync.dma_start(fn_v, fn.rearrange("(t p) d -> p t d", p=128))

        id_sb = const.tile([128, 128], BF16)
        nc.sync.dma_start(id_sb[:], ident)
        on_sb = const.tile([1, 128], BF16)
        nc.sync.dma_start(on_sb[:], ones1)

        ct_sb = const.tile([128, KT * 128], F32)  # natural centers [kpart, (j d)]
        ct_v = ct_sb[:].rearrange("p (j d) -> p j d", d=128)
        nc.sync.dma_start(ct_v, cn.rearrange("(j p) d -> p j d", p=128))

        # ---------------- center setup ----------------
        # q[p, j] = ||c_{j*128+p}||^2
        qw = setup.tile([128, KT], F32)
        for j in range(KT):
            sq_t = trash.tile([128, 128], F32, tag="sq_t")
            nc.scalar.activation(
                sq_t[:], ct_v[:, j, :], AF.Square, accum_out=qw[:, j : j + 1]
            )
        # rinv = 1/||c||  (reciprocal then sqrt)
        qinv = setup.tile([128, KT], F32)
        nc.vector.reciprocal(qinv[:], qw[:])
        rinv = setup.tile([128, KT], F32)
        nc.scalar.activation(rinv[:], qinv[:], AF.Sqrt)
        # r = q * rinv = ||c||;  lnr = ln ||c||
        rr = setup.tile([128, KT], F32)
        nc.vector.tensor_mul(rr[:], qw[:], rinv[:])
        lnr = setup.tile([128, KT], F32)
        nc.scalar.activation(lnr[:], rr[:], AF.Ln)
        lnr_bf = setup.tile([128, KT], BF16)
        nc.vector.tensor_copy(lnr_bf[:], lnr[:])
        # scaled normalizer: beta / ||c||
        rinv_b = setup.tile([128, KT], F32)
        nc.vector.tensor_scalar_mul(rinv_b[:], rinv[:], float(BETA))

        # c_hat_scaled tiles (natural layout) then PE-transpose into chT [d, k]
        chT_sb = const.tile([128, K], BF16)
        for j in range(KT):
            ch_j = setup.tile([128, 128], BF16, tag="ch_j")
            nc.vector.tensor_scalar_mul(ch_j[:], ct_v[:, j, :], rinv_b[:, j : j + 1])
            chT_ps = setup_ps.tile([128, 128], BF16, tag="chT_ps")
            nc.tensor.transpose(chT_ps[:], ch_j[:], id_sb[:])
            nc.scalar.activation(
                chT_sb[:, j * 128 : (j + 1) * 128], chT_ps[:], AF.Copy
            )

        # lnr_row [1, K] at partition 0 via column-wise PE transposes
        lnr_row = const.tile([1, K], BF16)
        for j in range(KT):
            rt_ps = setup_ps.tile([1, 128], BF16, tag="rt_ps")
            nc.tensor.transpose(rt_ps[:], lnr_bf[:, j : j + 1], id_sb[:])
            nc.scalar.activation(
                lnr_row[:, j * 128 : (j + 1) * 128], rt_ps[:], AF.Copy
            )

        setup_ps_cm.__exit__(None, None, None)
        gpool = ctx.enter_context(tc.tile_pool(name="gpool", bufs=3, space="PSUM"))

        # ---------------- wide accumulators ----------------
        negbm_w = const.tile([128, T], F32)
        sw = const.tile([128, T], F32)
        n2w = const.tile([128, T], F32)

        # ---------------- main loop (1-tile software pipeline) ----------------
        GN = 8  # norm-reduce batching group

        def emit_head(t, sq8):
            g_ps = gpool.tile([128, K], F32)
            lhs = ft_sb[:, t * 128 : (t + 1) * 128]
            nc.tensor.matmul(g_ps[:, 0:512], lhs, chT_sb[:, 0:512], start=True, stop=True)
            nc.tensor.matmul(g_ps[:, 512:1024], lhs, chT_sb[:, 512:1024], start=True, stop=True)
            nc.vector.reduce_max(
                negbm_w[:, t : t + 1], g_ps[:], axis=AX.X, negate=True
            )
            # copy bias to a rotating small tile: avoids a false cross-engine
            # WAR between ACT reading col t and DVE writing col t+1 of the
            # same wide tile (Tile deps are tile-granular).
            bias_sm = bpool.tile([128, 1], F32, tag="bias")
            nc.vector.tensor_copy(bias_sm[:], negbm_w[:, t : t + 1])
            # ||f||^2 partial: square on the (otherwise idle) GPSIMD engine
            nc.gpsimd.tensor_mul(
                sq8[:, (t % GN) * 128 : (t % GN + 1) * 128],
                fn_v[:, t, :], fn_v[:, t, :],
            )
            return g_ps, bias_sm

        def emit_tail(t, g_ps, bias_sm):
            nc.tensor.matmul(g_ps[:, 0:512], on_sb[:], lnr_row[:, 0:512], start=False, stop=True, skip_group_check=True)
            nc.tensor.matmul(g_ps[:, 512:1024], on_sb[:], lnr_row[:, 512:1024], start=False, stop=True, skip_group_check=True)
            e_sb = epool.tile([128, K], BF16)
            nc.scalar.activation(
                e_sb[:], g_ps[:], AF.Exp,
                bias=bias_sm[:], scale=1.0,
                accum_out=sw[:, t : t + 1],
            )

        def one_pass(_i=None):
            prev = None
            for g0 in range(0, T, GN):
                gn = min(GN, T - g0)
                sq8 = trash.tile([128, GN * 128], F32, tag="sq8")
                for t in range(g0, g0 + gn):
                    g_ps, bias_sm = emit_head(t, sq8)
                    if prev is not None:
                        emit_tail(*prev)
                    prev = (t, g_ps, bias_sm)
                nc.vector.reduce_sum(
                    n2w[:, g0 : g0 + gn],
                    sq8[:].rearrange("p (g d) -> p g d", d=128)[:, 0:gn, :],
                    axis=AX.X,
                )
            emit_tail(*prev)

        if rep == 1:
            one_pass()
        else:
            with tc.For_i(0, rep) as _i:
                one_pass(_i)

        # ---------------- epilogue: per-row loss ----------------
        m_w = setup.tile([128, T], F32)
        nc.vector.tensor_scalar_mul(m_w[:], negbm_w[:], -1.0 / BETA)
        n2i = setup.tile([128, T], F32)
        nc.vector.reciprocal(n2i[:], n2w[:])
        invn = setup.tile([128, T], F32)
        nc.scalar.activation(invn[:], n2i[:], AF.Sqrt)
        a_w = setup.tile([128, T], F32)
        nc.vector.tensor_mul(a_w[:], m_w[:], invn[:])
        b_w = setup.tile([128, T], F32)
        nc.vector.tensor_mul(b_w[:], a_w[:], sw[:])
        b2_w = setup.tile([128, T], F32)
        nc.vector.tensor_scalar_mul(b2_w[:], b_w[:], -2.0)
        r2_w = setup.tile([128, T], F32)
        nc.vector.tensor_mul(r2_w[:], sw[:], sw[:])
        t3_w = setup.tile([128, T], F32)
        nc.vector.tensor_add(t3_w[:], r2_w[:], b2_w[:])
        lw = setup.tile([128, T], F32)
        nc.vector.tensor_scalar_add(lw[:], t3_w[:], 1.0)
        nc.sync.dma_start(lossw, lw[:])

    nc.compile()
    _nc_cache[(rows_per_core, rep)] = nc
    return nc


def make_in_maps(features, centers, rows_per_core=ROWS_PER_CORE, n_cores=N_CORES):
    f_bf = features.astype(NP_BF16)
    shards = f_bf.reshape(n_cores, rows_per_core, D)
    ident = np.eye(128, dtype=NP_BF16)
    ones1 = np.ones((1, 128), dtype=NP_BF16)
    cns = np.ascontiguousarray(centers.astype(np.float32))
    in_maps = []
    for c in range(n_cores):
        s = shards[c]
        in_maps.append(
            {
                "ft": np.ascontiguousarray(s.T),
                "fn": np.ascontiguousarray(s),
                "cn": cns,
                "ident": ident,
                "ones1": ones1,
            }
        )
    return in_maps


def kernel(features, centers):
    features = np.asarray(features)
    centers = np.asarray(centers)
    nc = build_nc(ROWS_PER_CORE)
    in_maps = make_in_maps(features, centers)
    res = run_bass_kernel_spmd(nc, in_maps, core_ids=list(range(N_CORES)))
    total = 0.0
    for c in range(N_CORES):
        total += res.results[c]["lossw"].astype(np.float64).sum()
    return np.float32(total / (ROWS_PER_CORE * N_CORES))
